# revision 1
# baseline (speedup 1.0000x reference)
"""CAM (channel self-attention) kernel for Trainium2 — 8 NeuronCores, batch-parallel.

Math per batch element b (A = x[b] reshaped [N=4096, C=512]):
    G = A^T A                  [C, C]   (symmetric!)
    P = softmax_rows(G)        [C, C]
    Y = A P                    [N, C]
    out = gamma * Y + x

Sharding: data-parallel over batch — core i handles batch element i.
No cross-core communication needed.

Per-core schedule:
  - DMA x in 1 MiB groups -> A32 (f32, resident), cast to A16 (bf16).
  - Per 128-row chunk k, interleaved to keep the PE HAM-warm:
    cast -> 4 PE transposes (A^T blocks -> PSUM -> one strided copy to
    AT16) -> upper-triangle Gram matmuls (free dims 512/384/256/128,
    exploiting G's symmetry).
  - Lower triangle of G reconstructed with 6 PE transposes of the upper
    blocks after the Gram accumulation lands in SBUF.
  - softmax: DVE row-max (negated) -> ACT exp with fused row-sum -> DVE
    reciprocal -> DVE per-row scale, output bf16 P16.
  - Y = A P via PE: lhsT = AT16 tile, rhs = P16.
  - epilogue: one DVE scalar_tensor_tensor: out = (Y * gamma) + A32,
    staged in 512 KiB groups, DMA'd out.
"""

import numpy as np

import concourse.tile as tile
from concourse import bacc, mybir
from concourse.bass_utils import run_bass_kernel_spmd
from concourse.masks import make_identity

B = 8
H = 64
W = 64
C = 512
HW = H * W            # 4096 rows per batch element
NT = HW // 128        # 32 row chunks of 128
CT = C // 128         # 4 col chunks of 128
GRP = 4               # row chunks per input DMA group (1 MiB)
OGRP = 2              # row chunks per output DMA group (512 KiB)
ONG = NT // OGRP      # 16 output groups

F32 = mybir.dt.float32
BF16 = mybir.dt.bfloat16

_CACHE = {}


def _emit(nc, tc, out, x, gamma):
    from contextlib import ExitStack

    with ExitStack() as ctx:
        big = ctx.enter_context(tc.tile_pool(name="big", bufs=1))
        small = ctx.enter_context(tc.tile_pool(name="small", bufs=1))
        stat = ctx.enter_context(tc.tile_pool(name="stat", bufs=4))
        ostage = ctx.enter_context(tc.tile_pool(name="ostage", bufs=4))
        gps = ctx.enter_context(tc.tile_pool(name="gps", bufs=1, space="PSUM"))
        wps = ctx.enter_context(tc.tile_pool(name="wps", bufs=5, space="PSUM"))

        A32 = big.tile([128, NT, C], F32)     # x rows, n on partitions
        A16 = big.tile([128, NT, C], BF16)    # bf16 cast of A32
        AT16 = big.tile([128, CT, HW], BF16)  # A^T, c on partitions
        G32 = big.tile([128, CT, C], F32)     # full Gram matrix in SBUF
        E32 = big.tile([128, CT, C], F32)     # exp(G - rowmax)
        P16 = big.tile([128, CT, C], BF16)    # softmax(G) in bf16

        ident = small.tile([128, 128], BF16)
        make_identity(nc, ident[:])
        ident32 = small.tile([128, 128], F32)
        make_identity(nc, ident32[:])

        gB = small.tile([128, 1], F32)        # gamma broadcast to all partitions

        # PE warm-up: the HAM clock gate holds the PE at 1.2 GHz until it has
        # been busy ~3.4us. The PE is otherwise idle until the first input
        # chunk lands (~11us), so burn that window with dummy matmuls on a
        # zeroed scratch tile; real matmuls then start at 2.4 GHz.
        warm_src = small.tile([128, C], BF16)
        nc.gpsimd.memset(warm_src[:], 0.0)
        warm_ps = wps.tile([128, C], F32, name="warm", tag="w")
        for wi in range(30):
            nc.tensor.matmul(
                warm_ps[:], warm_src[:, 0:128], warm_src[:],
                start=(wi == 0), stop=(wi == 29),
            )

        # Upper-triangle Gram accumulators: G[mi-chunk, mi*128:].
        # g1 (384 cols) and g3 (128 cols) share one PSUM bank.
        g0 = gps.tile([128, C], F32, name="g0", tag="g0")
        g13 = gps.tile([128, C], F32, name="g13", tag="g13")
        g2 = gps.tile([128, C - 256], F32, name="g2", tag="g2")
        g_ps = [g0[:], g13[:, 0:384], g2[:], g13[:, 384:512]]

        # First loads chunk-granular so the PE can start early, then 1 MiB.
        load_groups = [1, 1, 2] + [GRP] * ((NT - 4) // GRP)
        assert sum(load_groups) == NT
        k0 = 0
        for gi, gsz in enumerate(load_groups):
            r0 = k0 * 128
            r1 = (k0 + gsz) * 128
            nc.sync.dma_start(
                A32[:, k0:k0 + gsz, :],
                x[r0:r1, :].rearrange("(t p) c -> p t c", p=128),
            )
            if gi == 0:
                # gamma: tiny load on the ACT HWDGE ring, off the input path
                nc.scalar.dma_start(gB[:], gamma[:])
            for j in range(gsz):
                k = k0 + j
                # cast f32 -> bf16 (DVE; keeps ACT free for A^T copies)
                nc.vector.tensor_copy(A16[:, k, :], A32[:, k, :])
                # A^T blocks of this chunk -> one PSUM bank, one strided copy
                tp = wps.tile([128, CT * 128], BF16, name="tp", tag="w")
                for ci in range(CT):
                    nc.tensor.transpose(
                        tp[:, ci * 128:(ci + 1) * 128],
                        A16[:, k, ci * 128:(ci + 1) * 128],
                        ident[:],
                    )
                nc.scalar.copy(
                    AT16[:, :, k * 128:(k + 1) * 128],
                    tp[:].rearrange("p (ci n) -> p ci n", ci=CT),
                )
                # upper-triangle Gram matmuls for this chunk
                for mi in range(CT):
                    nc.tensor.matmul(
                        g_ps[mi],
                        A16[:, k, mi * 128:(mi + 1) * 128],
                        A16[:, k, mi * 128:],
                        start=(k == 0),
                        stop=(k == NT - 1),
                        # g1/g3 share a bank; per-element has_written makes
                        # disjoint-region groups safe on HW
                        skip_group_check=(mi % 2 == 1),
                    )
            k0 += gsz

        # G (upper) PSUM -> SBUF
        for mi in range(CT):
            if mi % 2 == 0:
                nc.vector.tensor_copy(G32[:, mi, mi * 128:], g_ps[mi])
            else:
                nc.scalar.copy(G32[:, mi, mi * 128:], g_ps[mi])
        # reconstruct lower triangle: G[mi, j] = G[j, mi]^T for j < mi
        for mi in range(1, CT):
            for j in range(mi):
                lb = wps.tile([128, 128], F32, name="lb", tag="w")
                nc.tensor.transpose(
                    lb[:], G32[:, j, mi * 128:(mi + 1) * 128], ident32[:])
                if (mi + j) % 2 == 0:
                    nc.vector.tensor_copy(G32[:, mi, j * 128:(j + 1) * 128], lb[:])
                else:
                    nc.scalar.copy(G32[:, mi, j * 128:(j + 1) * 128], lb[:])

        # softmax over rows of G (free axis)
        for mi in range(CT):
            nmax = stat.tile([128, 1], F32)
            nc.vector.tensor_reduce(
                nmax[:], G32[:, mi, :],
                axis=mybir.AxisListType.X, op=mybir.AluOpType.max, negate=True,
            )
            esum = stat.tile([128, 1], F32)
            nc.scalar.activation(
                E32[:, mi, :], G32[:, mi, :],
                mybir.ActivationFunctionType.Exp,
                bias=nmax[:], scale=1.0, accum_out=esum[:],
            )
            rsum = stat.tile([128, 1], F32)
            nc.vector.reciprocal(rsum[:], esum[:])
            nc.vector.tensor_scalar_mul(P16[:, mi, :], E32[:, mi, :], rsum[:])

        # Y = A @ P, epilogue out = gamma * Y + x
        out_groups = [OGRP] * (ONG - 1) + [1, 1]
        t0 = 0
        for h, osz in enumerate(out_groups):
            r0 = t0 * 128
            r1 = (t0 + osz) * 128
            o32 = ostage.tile([128, OGRP, C], F32)
            for j in range(osz):
                t = t0 + j
                y = wps.tile([128, C], F32, name="y", tag="w")
                for ci in range(CT):
                    nc.tensor.matmul(
                        y[:],
                        AT16[:, ci, t * 128:(t + 1) * 128],
                        P16[:, ci, :],
                        start=(ci == 0),
                        stop=(ci == CT - 1),
                    )
                nc.vector.scalar_tensor_tensor(
                    o32[:, j, :], y[:], gB[:], A32[:, t, :],
                    op0=mybir.AluOpType.mult, op1=mybir.AluOpType.add,
                )
            # last groups ride the idle ACT ring to dodge Sync-ring backlog
            oeng = nc.scalar if h >= len(out_groups) - 2 else nc.sync
            oeng.dma_start(
                out[r0:r1, :].rearrange("(t p) c -> p t c", p=128),
                o32[:, 0:osz, :],
            )
            t0 += osz


def build():
    nc = bacc.Bacc("TRN2", target_bir_lowering=False, debug=False)
    x = nc.dram_tensor("x", [HW, C], F32, kind="ExternalInput").ap()
    gamma = nc.dram_tensor("gamma", [128, 1], F32, kind="ExternalInput").ap()
    out = nc.dram_tensor("out", [HW, C], F32, kind="ExternalOutput").ap()
    with tile.TileContext(nc) as tc:
        _emit(nc, tc, out, x, gamma)
    nc.compile()
    return nc


def kernel(x: np.ndarray, gamma: np.ndarray, trace: bool = False):
    assert x.shape == (B, H, W, C), x.shape
    if "nc" not in _CACHE:
        _CACHE["nc"] = build()
    nc = _CACHE["nc"]

    g128 = np.full((128, 1), np.float32(np.asarray(gamma).reshape(-1)[0]),
                   dtype=np.float32)
    in_maps = [
        {
            "x": np.ascontiguousarray(
                np.asarray(x[i], dtype=np.float32).reshape(HW, C)),
            "gamma": g128,
        }
        for i in range(B)
    ]
    if trace:
        res = run_bass_kernel_spmd(nc, in_maps, core_ids=list(range(B)),
                                   trace=True)
    else:
        # Force-untraced: a stray BASS_TRACE in the environment would route
        # through profiling hooks this image may not have.
        import os
        prev = os.environ.get("BASS_NEVER_TRACE")
        os.environ["BASS_NEVER_TRACE"] = "1"
        try:
            res = run_bass_kernel_spmd(nc, in_maps, core_ids=list(range(B)))
        finally:
            if prev is None:
                os.environ.pop("BASS_NEVER_TRACE", None)
            else:
                os.environ["BASS_NEVER_TRACE"] = prev
    _CACHE["last_result"] = res
    out = np.stack([res.results[i]["out"] for i in range(B)], axis=0)
    return out.reshape(B, H, W, C).astype(np.float32)



# revision 4
# speedup vs baseline: 1.0081x; 1.0081x over previous
"""CAM (channel self-attention) kernel for Trainium2 — 8 NeuronCores, batch-parallel.

Math per batch element b (A = x[b] reshaped [N=4096, C=512]):
    G = A^T A                  [C, C]
    P = softmax_rows(G)        [C, C]
    Y = A P                    [N, C]
    out = gamma * Y + x

Sharding: data-parallel over batch — core i handles batch element i.
No cross-core communication needed.

v2 design (vs bf16 baseline):
  - fp8e4 (E4M3) operands with MatmulPerfMode.DoubleRow: each matmul
    contracts TWO 128-row k-tiles per pass at 0.5 cycles/output-row,
    halving (or better) PE time for both the Gram and the Y = A P
    projection. Accumulation stays f32 in PSUM; the f32 epilogue
    (gamma * Y + x) preserves exactness of the residual path.
  - Partition-contiguous layout: x viewed as [128, 32, 512] via
    "(p t) c" — partition p holds rows 32p..32p+31, so a load/store
    group of g chunks is g*2KB contiguous per partition (few, large DMA
    descriptors; the in/out phases run at the HBM roofline).
  - Full Gram (all 4 row-blocks) accumulated in 4 PSUM banks; softmax
    reads straight from PSUM (no PSUM->SBUF copy of G, no
    lower-triangle reconstruction on the critical path).
  - A^T built per-chunk with 4 PE transposes (fp8) into AT8 laid out as
    [c-part, ci, k, j] so Y's stationary slices are contiguous.
  - Softmax: DVE row-max (negated, from PSUM) -> ACT exp with fused
    row-sum -> DVE reciprocal -> DVE per-row scale to fp8 P.
  - Epilogue: DVE scalar_tensor_tensor out = (Y * gamma) + x, staged in
    1 MiB groups, DMA'd out with 8KB/partition descriptors.
"""

import numpy as np

import concourse.tile as tile
from concourse import bacc, mybir
from concourse.bass_utils import run_bass_kernel_spmd
from concourse.masks import make_identity

B = 8
H = 64
W = 64
C = 512
HW = H * W            # 4096 rows per batch element
NT = HW // 128        # 32 row chunks of 128 (chunk k = rows {32p + k})
CT = C // 128         # 4 col chunks of 128

F32 = mybir.dt.float32
FP8 = mybir.dt.float8e4
DR = mybir.MatmulPerfMode.DoubleRow

_CACHE = {}


def _emit(nc, tc, out, x, gamma):
    from contextlib import ExitStack

    with ExitStack() as ctx:
        big = ctx.enter_context(tc.tile_pool(name="big", bufs=1))
        small = ctx.enter_context(tc.tile_pool(name="small", bufs=1))
        stat = ctx.enter_context(tc.tile_pool(name="stat", bufs=4))
        ostage = ctx.enter_context(tc.tile_pool(name="ostage", bufs=4))
        gps = ctx.enter_context(tc.tile_pool(name="gps", bufs=1, space="PSUM"))
        tps = ctx.enter_context(tc.tile_pool(name="tps", bufs=2, space="PSUM"))
        yps = ctx.enter_context(tc.tile_pool(name="yps", bufs=2, space="PSUM"))

        A32 = big.tile([128, NT, C], F32)       # x rows, row 32p+t on part p
        A8 = big.tile([128, NT, C], FP8)        # fp8 cast of A32
        AT8 = big.tile([128, CT, NT, 128], FP8)  # AT8[p,ci,k,j] = A[32j+k, 128ci+p]
        E32 = big.tile([128, CT, C], F32)       # exp(G - rowmax)
        P8 = big.tile([128, CT, C], FP8)        # softmax(G) in fp8

        ident8 = small.tile([128, 128], FP8)
        make_identity(nc, ident8[:])

        gB = small.tile([128, 1], F32)          # gamma broadcast to partitions

        # PE warm-up: HAM clock gate holds the PE at 1.2 GHz until it has
        # been busy ~3.4us; burn the DMA lead-in with dummy DoubleRow
        # matmuls so real matmuls run at 2.4 GHz in the right perf mode.
        warm8 = small.tile([128, 2, C], FP8)
        nc.gpsimd.memset(warm8[:], 0.0)
        warm_ps = yps.tile([128, C], F32, name="y", tag="y")
        NW = 24
        for wi in range(NW):
            nc.tensor.matmul(
                warm_ps[:], warm8[:, :, 0:128], warm8[:],
                start=(wi == 0), stop=(wi == NW - 1), perf_mode=DR,
            )

        # Gram accumulators: one full PSUM bank per 128-row block of G.
        g_ps = [gps.tile([128, C], F32, name=f"g{mi}", tag=f"g{mi}")
                for mi in range(CT)]

        xr = x.rearrange("(p t) c -> p t c", t=NT)

        # First groups small so PE work starts early, then 1 MiB groups.
        load_groups = [2, 2, 4, 4, 4, 4, 4, 4, 4]
        assert sum(load_groups) == NT
        k0 = 0
        for gi, gsz in enumerate(load_groups):
            nc.sync.dma_start(A32[:, k0:k0 + gsz, :], xr[:, k0:k0 + gsz, :])
            if gi == 0:
                # gamma: tiny load on the ACT HWDGE ring, off the input path
                nc.scalar.dma_start(gB[:], gamma[:])
            for j in range(gsz):
                k = k0 + j
                # cast f32 -> fp8 (DVE)
                nc.vector.tensor_copy(A8[:, k, :], A32[:, k, :])
                # A^T blocks of this chunk -> PSUM -> one strided copy (ACT).
                # fp8 transpose mode requires output element step of 2, so
                # tp carries a stride-2 layout dim.
                tp = tps.tile([128, CT * 128, 2], FP8, name="tp", tag="tp")
                for ci in range(CT):
                    nc.tensor.transpose(
                        tp[:, ci * 128:(ci + 1) * 128, 0],
                        A8[:, k, ci * 128:(ci + 1) * 128],
                        ident8[:],
                    )
                nc.scalar.copy(
                    AT8[:, :, k, :],
                    tp[:, :, 0].rearrange("p (ci n) -> p ci n", ci=CT),
                )
                # Gram: one DoubleRow matmul per row-block per chunk PAIR
                if k % 2 == 1:
                    kk = k - 1
                    for mi in range(CT):
                        nc.tensor.matmul(
                            g_ps[mi][:],
                            A8[:, kk:kk + 2, mi * 128:(mi + 1) * 128],
                            A8[:, kk:kk + 2, :],
                            start=(kk == 0), stop=(kk == NT - 2),
                            perf_mode=DR,
                        )
            k0 += gsz

        # softmax over rows of G (free axis), straight from PSUM
        for mi in range(CT):
            nmax = stat.tile([128, 1], F32)
            nc.vector.tensor_reduce(
                nmax[:], g_ps[mi][:],
                axis=mybir.AxisListType.X, op=mybir.AluOpType.max, negate=True,
            )
            esum = stat.tile([128, 1], F32)
            nc.scalar.activation(
                E32[:, mi, :], g_ps[mi][:],
                mybir.ActivationFunctionType.Exp,
                bias=nmax[:], scale=1.0, accum_out=esum[:],
            )
            rsum = stat.tile([128, 1], F32)
            nc.vector.reciprocal(rsum[:], esum[:])
            nc.vector.tensor_scalar_mul(P8[:, mi, :], E32[:, mi, :], rsum[:])

        # Y = A @ P (DoubleRow, 2 matmuls/chunk), epilogue gamma*Y + x
        out_r = out.rearrange("(p t) c -> p t c", t=NT)
        out_groups = [4] * 7 + [2, 1, 1]
        assert sum(out_groups) == NT
        t0 = 0
        for h, osz in enumerate(out_groups):
            o32 = ostage.tile([128, 4, C], F32)
            for j in range(osz):
                t = t0 + j
                y = yps.tile([128, C], F32, name="y", tag="y")
                for cp in range(CT // 2):
                    nc.tensor.matmul(
                        y[:],
                        AT8[:, 2 * cp:2 * cp + 2, t, :],
                        P8[:, 2 * cp:2 * cp + 2, :],
                        start=(cp == 0), stop=(cp == CT // 2 - 1),
                        perf_mode=DR,
                    )
                nc.vector.scalar_tensor_tensor(
                    o32[:, j, :], y[:], gB[:], A32[:, t, :],
                    op0=mybir.AluOpType.mult, op1=mybir.AluOpType.add,
                )
            # last groups ride the idle ACT ring to dodge Sync-ring backlog
            oeng = nc.scalar if h >= len(out_groups) - 2 else nc.sync
            oeng.dma_start(out_r[:, t0:t0 + osz, :], o32[:, 0:osz, :])
            t0 += osz


def build():
    nc = bacc.Bacc("TRN2", target_bir_lowering=False, debug=False)
    x = nc.dram_tensor("x", [HW, C], F32, kind="ExternalInput").ap()
    gamma = nc.dram_tensor("gamma", [128, 1], F32, kind="ExternalInput").ap()
    out = nc.dram_tensor("out", [HW, C], F32, kind="ExternalOutput").ap()
    with tile.TileContext(nc) as tc:
        _emit(nc, tc, out, x, gamma)
    nc.compile()
    return nc


def kernel(x: np.ndarray, gamma: np.ndarray, trace: bool = False):
    assert x.shape == (B, H, W, C), x.shape
    if "nc" not in _CACHE:
        _CACHE["nc"] = build()
    nc = _CACHE["nc"]

    g128 = np.full((128, 1), np.float32(np.asarray(gamma).reshape(-1)[0]),
                   dtype=np.float32)
    in_maps = [
        {
            "x": np.ascontiguousarray(
                np.asarray(x[i], dtype=np.float32).reshape(HW, C)),
            "gamma": g128,
        }
        for i in range(B)
    ]
    if trace:
        res = run_bass_kernel_spmd(nc, in_maps, core_ids=list(range(B)),
                                   trace=True)
    else:
        # Force-untraced: a stray BASS_TRACE in the environment would route
        # through profiling hooks this image may not have.
        import os
        prev = os.environ.get("BASS_NEVER_TRACE")
        os.environ["BASS_NEVER_TRACE"] = "1"
        try:
            res = run_bass_kernel_spmd(nc, in_maps, core_ids=list(range(B)))
        finally:
            if prev is None:
                os.environ.pop("BASS_NEVER_TRACE", None)
            else:
                os.environ["BASS_NEVER_TRACE"] = prev
    _CACHE["last_result"] = res
    out = np.stack([res.results[i]["out"] for i in range(B)], axis=0)
    return out.reshape(B, H, W, C).astype(np.float32)


# revision 9
# speedup vs baseline: 1.0249x; 1.0167x over previous
"""CAM (channel self-attention) kernel for Trainium2 — 8 NeuronCores, batch-parallel.

Math per batch element b (A = x[b] reshaped [N=4096, C=512]):
    G = A^T A                  [C, C]
    P = softmax_rows(G)        [C, C]
    Y = A P                    [N, C]
    out = gamma * Y + x

Sharding: data-parallel over batch — core i handles batch element i.
No cross-core communication needed.

v2 design (vs bf16 baseline):
  - fp8e4 (E4M3) operands with MatmulPerfMode.DoubleRow: each matmul
    contracts TWO 128-row k-tiles per pass at 0.5 cycles/output-row,
    halving (or better) PE time for both the Gram and the Y = A P
    projection. Accumulation stays f32 in PSUM; the f32 epilogue
    (gamma * Y + x) preserves exactness of the residual path.
  - Partition-contiguous layout: x viewed as [128, 32, 512] via
    "(p t) c" — partition p holds rows 32p..32p+31, so a load/store
    group of g chunks is g*2KB contiguous per partition (few, large DMA
    descriptors; the in/out phases run at the HBM roofline).
  - Full Gram (all 4 row-blocks) accumulated in 4 PSUM banks; softmax
    reads straight from PSUM (no PSUM->SBUF copy of G, no
    lower-triangle reconstruction on the critical path).
  - A^T built per-chunk with 4 PE transposes (fp8) into AT8 laid out as
    [c-part, ci, k, j] so Y's stationary slices are contiguous.
  - Softmax: DVE row-max (negated, from PSUM) -> ACT exp with fused
    row-sum -> DVE reciprocal -> DVE per-row scale to fp8 P.
  - Epilogue: DVE scalar_tensor_tensor out = (Y * gamma) + x, staged in
    1 MiB groups, DMA'd out with 8KB/partition descriptors.
"""

import numpy as np

import concourse.tile as tile
from concourse import bacc, mybir
from concourse.bass_utils import run_bass_kernel_spmd
from concourse.masks import make_identity

B = 8
H = 64
W = 64
C = 512
HW = H * W            # 4096 rows per batch element
NT = HW // 128        # 32 row chunks of 128 (chunk k = rows {32p + k})
CT = C // 128         # 4 col chunks of 128

F32 = mybir.dt.float32
FP8 = mybir.dt.float8e4
DR = mybir.MatmulPerfMode.DoubleRow

_CACHE = {}


def _emit(nc, tc, out, x, gamma):
    from contextlib import ExitStack

    with ExitStack() as ctx:
        big = ctx.enter_context(tc.tile_pool(name="big", bufs=1))
        small = ctx.enter_context(tc.tile_pool(name="small", bufs=1))
        stat = ctx.enter_context(tc.tile_pool(name="stat", bufs=4))
        ostage = ctx.enter_context(tc.tile_pool(name="ostage", bufs=4))
        gps = ctx.enter_context(tc.tile_pool(name="gps", bufs=1, space="PSUM"))
        tps = ctx.enter_context(tc.tile_pool(name="tps", bufs=2, space="PSUM"))
        yps = ctx.enter_context(tc.tile_pool(name="yps", bufs=2, space="PSUM"))

        A32 = big.tile([128, NT, C], F32)       # x rows, row 32p+t on part p
        A8 = big.tile([128, NT, C], FP8)        # fp8 cast of A32
        # A^T, stride-2 padded (fp8 PE transposes write with element step 2;
        # keeping the pad lets the PSUM->SBUF copy run as contiguous u16):
        # AT8[p, ci, k, j, 0] = A[32j+k, 128ci+p]
        AT8 = big.tile([128, CT, NT, 128, 2], FP8)
        E32 = big.tile([128, CT, C], F32)       # exp(G - rowmax)
        P8 = big.tile([128, CT, C], FP8)        # softmax(G) in fp8

        ident8 = small.tile([128, 128], FP8)
        make_identity(nc, ident8[:])

        gB = small.tile([128, 1], F32)          # gamma broadcast to partitions

        # PE warm-up: HAM clock gate holds the PE at 1.2 GHz until it has
        # been busy ~3.4us; burn the DMA lead-in with dummy DoubleRow
        # matmuls so real matmuls run at 2.4 GHz in the right perf mode.
        warm8 = small.tile([128, 2, C], FP8)
        nc.gpsimd.memset(warm8[:], 0.0)
        warm_ps = yps.tile([128, C], F32, name="y", tag="y")
        NW = 16
        for wi in range(NW):
            nc.tensor.matmul(
                warm_ps[:, 0:256], warm8[:, :, 0:128], warm8[:, :, 0:256],
                start=(wi == 0), stop=(wi == NW - 1), perf_mode=DR,
            )

        # Gram accumulators: one full PSUM bank per 128-row block of G.
        g_ps = [gps.tile([128, C], F32, name=f"g{mi}", tag=f"g{mi}")
                for mi in range(CT)]

        xr = x.rearrange("(p t) c -> p t c", t=NT)

        # First groups small so PE work starts early, then 1 MiB groups.
        load_groups = [1, 1, 2, 4, 4, 4, 4, 4, 4, 4]
        assert sum(load_groups) == NT
        k0 = 0
        for gi, gsz in enumerate(load_groups):
            nc.sync.dma_start(A32[:, k0:k0 + gsz, :], xr[:, k0:k0 + gsz, :])
            if gi == 0:
                # gamma: tiny load on the ACT HWDGE ring, off the input path
                nc.scalar.dma_start(gB[:], gamma[:])
            for j in range(gsz):
                k = k0 + j
                # cast f32 -> fp8 (DVE)
                nc.vector.tensor_copy(A8[:, k, :], A32[:, k, :])
                # A^T blocks of this chunk -> PSUM -> one u16 copy (ACT).
                # fp8 transpose mode writes with element step 2; the pad dim
                # stays in place so the copy is a contiguous 16-bit move.
                # (middle "2" pads the tile to a full 2KB PSUM bank)
                tp = tps.tile([128, CT, 2, 128, 2], FP8, name="tp", tag="tp")
                for ci in range(CT):
                    nc.tensor.transpose(
                        tp[:, ci, 0, :, 0],
                        A8[:, k, ci * 128:(ci + 1) * 128],
                        ident8[:],
                    )
                nc.scalar.copy(
                    AT8[:, :, k, :, :].bitcast(mybir.dt.uint16),
                    tp[:, :, 0, :, :].bitcast(mybir.dt.uint16),
                )
                # Gram: one DoubleRow matmul per row-block per chunk PAIR
                if k % 2 == 1:
                    kk = k - 1
                    for mi in range(CT):
                        nc.tensor.matmul(
                            g_ps[mi][:],
                            A8[:, kk:kk + 2, mi * 128:(mi + 1) * 128],
                            A8[:, kk:kk + 2, :],
                            start=(kk == 0), stop=(kk == NT - 2),
                            perf_mode=DR,
                        )
            k0 += gsz

        # softmax over rows of G (free axis), straight from PSUM
        for mi in range(CT):
            nmax = stat.tile([128, 1], F32)
            nc.vector.tensor_reduce(
                nmax[:], g_ps[mi][:],
                axis=mybir.AxisListType.X, op=mybir.AluOpType.max, negate=True,
            )
            esum = stat.tile([128, 1], F32)
            nc.scalar.activation(
                E32[:, mi, :], g_ps[mi][:],
                mybir.ActivationFunctionType.Exp,
                bias=nmax[:], scale=1.0, accum_out=esum[:],
            )
            rsum = stat.tile([128, 1], F32)
            nc.vector.reciprocal(rsum[:], esum[:])
            nc.vector.tensor_scalar_mul(P8[:, mi, :], E32[:, mi, :], rsum[:])

        # Y = A @ P (DoubleRow, 2 matmuls/chunk), epilogue gamma*Y + x.
        # Chunks are processed in pairs with their matmuls interleaved
        # across the two PSUM y banks so weight loads overlap streaming.
        out_r = out.rearrange("(p t) c -> p t c", t=NT)
        out_groups = [4] * 7 + [2, 2]
        assert sum(out_groups) == NT
        t0 = 0
        for h, osz in enumerate(out_groups):
            o32 = ostage.tile([128, 4, C], F32)
            for j in range(0, osz, 2):
                t = t0 + j
                ya = yps.tile([128, C], F32, name="y", tag="y")
                yb = yps.tile([128, C], F32, name="y", tag="y")
                for cp in range(CT // 2):
                    nc.tensor.matmul(
                        ya[:],
                        AT8[:, 2 * cp:2 * cp + 2, t, :, 0],
                        P8[:, 2 * cp:2 * cp + 2, :],
                        start=(cp == 0), stop=(cp == CT // 2 - 1),
                        perf_mode=DR,
                    )
                    nc.tensor.matmul(
                        yb[:],
                        AT8[:, 2 * cp:2 * cp + 2, t + 1, :, 0],
                        P8[:, 2 * cp:2 * cp + 2, :],
                        start=(cp == 0), stop=(cp == CT // 2 - 1),
                        perf_mode=DR,
                    )
                nc.vector.scalar_tensor_tensor(
                    o32[:, j, :], ya[:], gB[:], A32[:, t, :],
                    op0=mybir.AluOpType.mult, op1=mybir.AluOpType.add,
                )
                nc.vector.scalar_tensor_tensor(
                    o32[:, j + 1, :], yb[:], gB[:], A32[:, t + 1, :],
                    op0=mybir.AluOpType.mult, op1=mybir.AluOpType.add,
                )
            # last groups ride the idle ACT ring to dodge Sync-ring backlog
            oeng = nc.scalar if h >= len(out_groups) - 2 else nc.sync
            oeng.dma_start(out_r[:, t0:t0 + osz, :], o32[:, 0:osz, :])
            t0 += osz


def build():
    nc = bacc.Bacc("TRN2", target_bir_lowering=False, debug=False)
    x = nc.dram_tensor("x", [HW, C], F32, kind="ExternalInput").ap()
    gamma = nc.dram_tensor("gamma", [128, 1], F32, kind="ExternalInput").ap()
    out = nc.dram_tensor("out", [HW, C], F32, kind="ExternalOutput").ap()
    with tile.TileContext(nc) as tc:
        _emit(nc, tc, out, x, gamma)
    nc.compile()
    return nc


def kernel(x: np.ndarray, gamma: np.ndarray, trace: bool = False):
    assert x.shape == (B, H, W, C), x.shape
    if "nc" not in _CACHE:
        _CACHE["nc"] = build()
    nc = _CACHE["nc"]

    g128 = np.full((128, 1), np.float32(np.asarray(gamma).reshape(-1)[0]),
                   dtype=np.float32)
    in_maps = [
        {
            "x": np.ascontiguousarray(
                np.asarray(x[i], dtype=np.float32).reshape(HW, C)),
            "gamma": g128,
        }
        for i in range(B)
    ]
    if trace:
        res = run_bass_kernel_spmd(nc, in_maps, core_ids=list(range(B)),
                                   trace=True)
    else:
        # Force-untraced: a stray BASS_TRACE in the environment would route
        # through profiling hooks this image may not have.
        import os
        prev = os.environ.get("BASS_NEVER_TRACE")
        os.environ["BASS_NEVER_TRACE"] = "1"
        try:
            res = run_bass_kernel_spmd(nc, in_maps, core_ids=list(range(B)))
        finally:
            if prev is None:
                os.environ.pop("BASS_NEVER_TRACE", None)
            else:
                os.environ["BASS_NEVER_TRACE"] = prev
    _CACHE["last_result"] = res
    out = np.stack([res.results[i]["out"] for i in range(B)], axis=0)
    return out.reshape(B, H, W, C).astype(np.float32)
